# revision 1
# baseline (speedup 1.0000x reference)
"""Trainium2 Bass kernel for nn_ContextEncoder (4-head GlobalAttention pooling).

Strategy:
  - Shard the 256 graphs into 8 contiguous shards of 32 (batch is sorted, so
    each shard is a contiguous node range) -> data-parallel over graphs, no
    cross-core reduction needed.
  - Softmax normalization is deferred: accumulate s1[b,k,:] = sum_n e_nk *
    relu(h1_nk) and den[b,k] = sum_n e_nk on device; normalize + apply the
    second value-layer matmul (nn_w2, which commutes with the segment sum)
    + nn_b2 on the host in f32.
  - gate_b1/nn_b1 are folded into the matmuls via a ones-row appended to x^T.
  - nn_b2 is folded out of the segment sum entirely (gates sum to 1).
"""

import sys

sys.path.insert(0, "/opt/trn_rl_repo")

import numpy as np
import ml_dtypes

import concourse.bass as bass
import concourse.bacc as bacc
import concourse.mybir as mybir
from concourse.tile import TileContext
from concourse.bass_utils import run_bass_kernel_spmd

BF16 = ml_dtypes.bfloat16

N_POOL = 4
DIM_EMB = 128
DIM_HID = 128
FIRST_DIM = 134
N_GRAPHS = 256
NCORES = 8
GPC = N_GRAPHS // NCORES  # graphs per core
NT = 512  # nodes per tile (one PSUM bank of fp32)

_cache: dict = {}

# Set by kernel() when TRN_BASS_TRACE env is set; read by test.py.
last_exec_time_ns = None
last_results = None


TRACE_SIM = False  # set True to publish a cost-model (scheduling) perfetto trace


def _build(nt_pad: int, reps: int = 1):
    """Build + compile the 8-core SPMD Bass program for a padded shard of
    nt_pad nodes. Returns the compiled Bacc object. reps>1 repeats the tile
    loop (timing amplification only; results are then wrong)."""
    F32 = mybir.dt.float32
    BF = mybir.dt.bfloat16
    T = nt_pad // NT

    nc = bacc.Bacc("TRN2", target_bir_lowering=False, debug=False, num_devices=NCORES)

    XM = nc.dram_tensor("xm", [128, nt_pad], BF, kind="ExternalInput")
    XR = nc.dram_tensor("xr", [8, nt_pad], BF, kind="ExternalInput")
    IND = nc.dram_tensor("ind", [nt_pad, GPC], BF, kind="ExternalInput")
    WGM = nc.dram_tensor("wgm", [128, 512], BF, kind="ExternalInput")
    WGR = nc.dram_tensor("wgr", [8, 512], BF, kind="ExternalInput")
    WNM = nc.dram_tensor("wnm", [128, 512], BF, kind="ExternalInput")
    WNR = nc.dram_tensor("wnr", [8, 512], BF, kind="ExternalInput")
    W2S = nc.dram_tensor("w2s", [128, 64], BF, kind="ExternalInput")
    B2 = nc.dram_tensor("b2", [16, 1], F32, kind="ExternalInput")
    S1 = nc.dram_tensor("s1", [48, 512], F32, kind="ExternalOutput")

    Relu = mybir.ActivationFunctionType.Relu
    Exp = mybir.ActivationFunctionType.Exp
    Mult = mybir.AluOpType.mult

    with TileContext(nc, trace_sim=TRACE_SIM) as tc:
        with (
            tc.tile_pool(name="consts", bufs=1) as consts,
            tc.tile_pool(name="xin", bufs=3) as xin,
            tc.tile_pool(name="work", bufs=3) as work,
            tc.tile_pool(name="outp", bufs=1) as outp,
            tc.tile_pool(name="ps_g", bufs=4, space="PSUM") as ps_g,
            tc.tile_pool(name="ps_n", bufs=2, space="PSUM") as ps_n,
            tc.tile_pool(name="ps_s", bufs=1, space="PSUM") as ps_s,
            tc.tile_pool(name="ps_acc", bufs=1, space="PSUM") as ps_acc,
        ):
            # --- constants (loaded once) ---
            wgm = consts.tile([128, N_POOL, 128], BF)
            nc.sync.dma_start(out=wgm, in_=WGM.ap().rearrange("p (k h) -> p k h", k=N_POOL))
            # remainder gate weights replicated at partition offsets 32k so the
            # 4 K=8 remainder matmuls can run concurrently in distinct PE
            # row-groups (tile_position=(32k, 0)).
            wgrp = consts.tile([128, 128], BF)
            for k in range(N_POOL):
                nc.sync.dma_start(
                    out=wgrp[32 * k : 32 * k + 8, :],
                    in_=WGR[:, 128 * k : 128 * (k + 1)],
                )
            wnm = consts.tile([128, 512], BF)
            nc.sync.dma_start(out=wnm, in_=WNM[:, :])
            # nn remainder weights replicated at partition offsets 0/32 for
            # 2-way row-group packing.
            wnrp = consts.tile([64, 512], BF)
            for r in range(2):
                nc.sync.dma_start(out=wnrp[32 * r : 32 * r + 8, :], in_=WNR[:, :])
            w2s = consts.tile([128, N_POOL, 16], BF)
            nc.sync.dma_start(out=w2s, in_=W2S.ap().rearrange("p (k j) -> p k j", k=N_POOL))
            b2sb = consts.tile([16, 1], F32)
            nc.sync.dma_start(out=b2sb, in_=B2[:, :])
            zero48 = consts.tile([128, 48], BF)
            nc.vector.memset(zero48, 0.0)

            # --- persistent accumulator: rows 0:32 = pooled s1, rows 32:48 =
            # denominators (written via col-group 1). One zeroing matmul sets
            # has_written for the whole region so every real matmul can be
            # start=False (order-independent accumulation).
            pool_ps = ps_acc.tile([48, 512], F32)
            nc.tensor.matmul(
                pool_ps, zero48, wnm, start=True, stop=False, skip_group_check=True
            )

            for rep in range(reps):
              for t in range(T):
                last = (t == T - 1) and (rep == reps - 1)
                n0 = t * NT

                xm = xin.tile([128, NT], BF, tag="xm")
                nc.sync.dma_start(out=xm, in_=XM[:, n0 : n0 + NT])
                # xr replicated into partition rows 32r:32r+8 for row-group packing
                xrp = xin.tile([128, NT], BF, tag="xrp")
                for g in range(4):
                    nc.sync.dma_start(
                        out=xrp[32 * g : 32 * g + 8, :], in_=XR[:, n0 : n0 + NT]
                    )
                ind = xin.tile([128, 4, GPC], BF, tag="ind")
                nc.sync.dma_start(
                    out=ind,
                    in_=IND[n0 : n0 + NT, :].rearrange("(blk p) b -> p blk b", p=128),
                )

                # --- gate path: [h, node] orientation, weights stationary ---
                psum_s = ps_s.tile([16, NT], F32, tag="psum_s")
                pgs = []
                for k in range(N_POOL):
                    pg = ps_g.tile([128, NT], F32, tag="pg")
                    nc.tensor.matmul(pg, wgm[:, k, :], xm, start=True, stop=False)
                    pgs.append(pg)
                for k in range(N_POOL):
                    nc.tensor.matmul(
                        pgs[k],
                        wgrp[32 * k : 32 * k + 8, :],
                        xrp[32 * k : 32 * k + 8, :],
                        start=False,
                        stop=True,
                        tile_position=(32 * k, 0),
                    )
                for k in range(N_POOL):
                    rg = work.tile([128, NT], BF, tag="rg")
                    if k % 2 == 0:
                        nc.scalar.activation(rg, pgs[k], Relu)
                    else:
                        nc.vector.tensor_scalar_max(rg, pgs[k], 0.0)
                    # score row k via zero-masked w2 stack, accumulated into one bank
                    nc.tensor.matmul(
                        psum_s,
                        w2s[:, k, :],
                        rg,
                        start=(k == 0),
                        stop=(k == N_POOL - 1),
                        skip_group_check=True,
                    )

                e_sb = work.tile([16, NT], BF, tag="e_sb")
                nc.scalar.activation(e_sb, psum_s, Exp, bias=b2sb[:, :])

                # --- nn path + pooling, per 128-node block ---
                pns = []
                for i in range(4):
                    blk = slice(i * 128, (i + 1) * 128)
                    pn = ps_n.tile([128, 512], F32, tag="pn")
                    nc.tensor.matmul(pn, xm[:, blk], wnm, start=True, stop=False)
                    r = i % 2
                    nc.tensor.matmul(
                        pn,
                        xrp[32 * r : 32 * r + 8, blk],
                        wnrp[32 * r : 32 * r + 8, :],
                        start=False,
                        stop=True,
                        tile_position=(32 * r, 0),
                    )
                    pns.append(pn)

                    h1 = work.tile([128, 512], BF, tag="h1")
                    if i % 2 == 0:
                        nc.vector.tensor_scalar_max(h1, pn, 0.0)
                    else:
                        nc.scalar.activation(h1, pn, Relu)

                    eT = work.tile([128, 16], BF, tag="eT")
                    nc.sync.dma_start_transpose(eT, e_sb[:, blk])

                    e_ind = work.tile([128, N_POOL, GPC], BF, tag="e_ind")
                    nc.vector.tensor_tensor(
                        e_ind,
                        ind[:, i, None, :].to_broadcast([128, N_POOL, GPC]),
                        eT[:, 0:N_POOL, None].to_broadcast([128, N_POOL, GPC]),
                        Mult,
                    )

                    for k in range(N_POOL):
                        nc.tensor.matmul(
                            pool_ps[0:GPC, k * 128 : (k + 1) * 128],
                            e_ind[:, k, :],
                            h1[:, k * 128 : (k + 1) * 128],
                            start=False,
                            stop=(last and i == 3 and k == N_POOL - 1),
                            skip_group_check=True,
                        )
                    # denominators into pool rows 32:48 (col-group 1)
                    nc.tensor.matmul(
                        pool_ps[32:48, 0:GPC],
                        eT,
                        ind[:, i, :],
                        start=False,
                        stop=(last and i == 3),
                        skip_group_check=True,
                        tile_position=(0, 32),
                    )

            # --- evacuate accumulator ---
            s1_sb = outp.tile([48, 512], F32)
            nc.vector.tensor_copy(s1_sb, pool_ps)
            nc.sync.dma_start(out=S1[:, :], in_=s1_sb)

    nc.compile()
    return nc


def kernel(**inputs) -> np.ndarray:
    global last_exec_time_ns, last_results
    import os

    x = np.asarray(inputs["x"], dtype=np.float32)  # [N, 134]
    batch = np.asarray(inputs["batch"]).astype(np.int64)  # [N], sorted
    n_nodes = np.asarray(inputs["n_nodes"], dtype=np.float32)
    gate_w1 = np.asarray(inputs["gate_w1"], dtype=np.float32)  # [4,134,128]
    gate_b1 = np.asarray(inputs["gate_b1"], dtype=np.float32)  # [4,128]
    gate_w2 = np.asarray(inputs["gate_w2"], dtype=np.float32)  # [4,128]
    gate_b2 = np.asarray(inputs["gate_b2"], dtype=np.float32)  # [4]
    nn_w1 = np.asarray(inputs["nn_w1"], dtype=np.float32)  # [4,134,128]
    nn_b1 = np.asarray(inputs["nn_b1"], dtype=np.float32)  # [4,128]
    nn_w2 = np.asarray(inputs["nn_w2"], dtype=np.float32)  # [4,128,128]
    nn_b2 = np.asarray(inputs["nn_b2"], dtype=np.float32)  # [4,128]

    N = x.shape[0]
    B = N_GRAPHS

    counts = np.bincount(batch, minlength=B)
    bounds = np.concatenate([[0], np.cumsum(counts)])  # [B+1]
    core_start = bounds[np.arange(NCORES + 1) * GPC]  # [9]
    shard_sizes = np.diff(core_start)
    nt_pad = int(-(-max(shard_sizes.max(), 1) // 2048) * 2048)

    # --- shared (replicated) weight arrays ---
    def pack_w1(w1, b1):
        main = np.ascontiguousarray(
            w1[:, :128, :].transpose(1, 0, 2).reshape(128, 512)
        ).astype(BF16)
        rem = np.zeros((8, 512), dtype=BF16)
        rem[:6] = w1[:, 128:134, :].transpose(1, 0, 2).reshape(6, 512).astype(BF16)
        rem[6] = b1.reshape(512).astype(BF16)
        return main, rem

    wgm_h, wgr_h = pack_w1(gate_w1, gate_b1)
    wnm_h, wnr_h = pack_w1(nn_w1, nn_b1)
    w2s_h = np.zeros((128, 64), dtype=BF16)
    for k in range(N_POOL):
        w2s_h[:, 16 * k + k] = gate_w2[k].astype(BF16)
    b2_h = np.zeros((16, 1), dtype=np.float32)
    b2_h[:N_POOL, 0] = gate_b2

    # --- per-core inputs ---
    in_maps = []
    for c in range(NCORES):
        s, e = int(core_start[c]), int(core_start[c + 1])
        n = e - s
        xm = np.zeros((128, nt_pad), dtype=BF16)
        xm[:, :n] = x[s:e, :128].T.astype(BF16)
        xr = np.zeros((8, nt_pad), dtype=BF16)
        xr[:6, :n] = x[s:e, 128:134].T.astype(BF16)
        xr[6, :n] = 1.0
        ind = np.zeros((nt_pad, GPC), dtype=BF16)
        if n > 0:
            ind[np.arange(n), batch[s:e] - c * GPC] = 1.0
        in_maps.append(
            {
                "xm": xm,
                "xr": xr,
                "ind": ind,
                "wgm": wgm_h,
                "wgr": wgr_h,
                "wnm": wnm_h,
                "wnr": wnr_h,
                "w2s": w2s_h,
                "b2": b2_h,
            }
        )

    if nt_pad not in _cache:
        _cache[nt_pad] = _build(nt_pad)
    nc = _cache[nt_pad]

    trace = bool(os.environ.get("TRN_BASS_TRACE"))
    try:
        res = run_bass_kernel_spmd(
            nc, in_maps, core_ids=list(range(NCORES)), trace=trace
        )
    except ModuleNotFoundError:
        res = run_bass_kernel_spmd(
            nc, in_maps, core_ids=list(range(NCORES)), trace=False
        )
    last_exec_time_ns = res.exec_time_ns
    last_results = res

    # --- host-side finish (all f32) ---
    raw = [np.asarray(res.results[c]["s1"], np.float32) for c in range(NCORES)]
    s1 = np.stack([r[:GPC] for r in raw])
    den = np.stack([r[32 : 32 + N_POOL, :GPC] for r in raw])  # [8, 4, 32]
    s1 = s1.reshape(NCORES, GPC, N_POOL, DIM_HID)  # [8,32,4,128]
    den = den.transpose(0, 2, 1)  # [8,32,4]
    den_safe = np.where(den == 0.0, 1.0, den)
    g1 = s1 / den_safe[..., None]  # normalized gated hidden sums
    pooled = np.einsum("cgkh,khd->cgkd", g1, nn_w2) + nn_b2  # [8,32,4,128]
    nonempty = (counts.reshape(NCORES, GPC) > 0).astype(np.float32)
    pooled *= nonempty[:, :, None, None]
    ctx = pooled.reshape(B, N_POOL * DIM_EMB)

    extras = [
        np.asarray(inputs[k], dtype=np.float32)
        for k in [
            "n_nodes",
            "Omegas",
            "Phis",
            "Lambdas",
            "Omegas_norm",
            "Phis_norm",
            "Lambdas_norm",
        ]
    ]
    return np.concatenate([ctx] + extras, axis=1).astype(np.float32)



# revision 55
# speedup vs baseline: 1.3946x; 1.3946x over previous
"""Trainium2 Bass kernel for nn_ContextEncoder (4-head GlobalAttention pooling).

Strategy (v3, hardware-legal):
  - 8 contiguous graph-shards (batch sorted) -> data-parallel, no collectives.
  - Main x@W1 matmuls in bf16, two passes (features 0:128, then the 6
    remainder features + bias row as a 7-row pass).
  - Scores via the data-stationary trick: stationary = relu(gate hidden)
    block [128h x 128n], moving = a single w2 column -> out [128 nodes, 1].
    Nearly free on the PE (cost scales with moving free size).
  - Softmax denominators and gated segment-sum pooling accumulate in ONE
    persistent PSUM bank across all tiles (start=False matmuls onto a
    one-time-zeroed bank).  Scores live in the same bank in a 2-slot
    ping-pong region, recycled by PE matmuls with negated w2 (exact
    cancellation), so no engine has to memset PSUM.
  - Pooling contracts 256 nodes per pass via fp8 DoubleRow (stationary =
    h1 block-pair fp8, moving = e*ind block-pair fp8).  DoubleRow
    destinations must start at partition 0, so the accumulator keeps
    kh = slice*64 + partition with only partitions 0:64 used.
  - exp is split per head-pair so each PE->scalar score roundtrip overlaps
    the tile boundary; e*ind products run on gpsimd (SBUF-only there).
  - gate relus on the scalar engine, nn relus on DVE, e*ind on gpsimd:
    scalar ~2.4us, DVE ~2.6us, Pool ~1.3us, PE ~3.6us per 512-node tile.
  - Host finishes with the nn_w2 matmul (commutes with the segment sum)
    and the softmax normalization, in f32.
"""

import sys

sys.path.insert(0, "/opt/trn_rl_repo")

import numpy as np
import ml_dtypes

import concourse.bass as bass
import concourse.bacc as bacc
import concourse.mybir as mybir
from concourse.tile import TileContext
from concourse.bass_utils import run_bass_kernel_spmd

BF16 = ml_dtypes.bfloat16
F8 = ml_dtypes.float8_e4m3

N_POOL = 4
DIM_EMB = 128
DIM_HID = 128
FIRST_DIM = 134
N_GRAPHS = 256
NCORES = 8
GPC = N_GRAPHS // NCORES  # graphs per core
NT = 512  # nodes per tile

_cache: dict = {}

last_exec_time_ns = None
last_results = None
last_sim_ns = None

DR = mybir.MatmulPerfMode.DoubleRow
Relu = mybir.ActivationFunctionType.Relu
Exp = mybir.ActivationFunctionType.Exp
Mult = mybir.AluOpType.mult
Max = mybir.AluOpType.max

# engine assignment for balance-critical ops: "S"=scalar, "V"=DVE
# (gpsimd cannot touch PSUM on TRN2, so PSUM-reading relus are S/V only)
CFG = {
    "gA": "S", "gB": "S",
    "b0": "V", "b1": "V", "b2": "V", "b3": "V",
    "neg_clear": True,
}


def _build(nt_pad: int, cfg: dict | None = None):
    cfg = dict(CFG if cfg is None else cfg)
    F32 = mybir.dt.float32
    BF = mybir.dt.bfloat16
    E4 = mybir.dt.float8e4
    T = nt_pad // NT

    nc = bacc.Bacc("TRN2", target_bir_lowering=False, debug=False, num_devices=NCORES)

    XM = nc.dram_tensor("xm", [128, nt_pad], BF, kind="ExternalInput")
    XR = nc.dram_tensor("xr", [7, nt_pad], BF, kind="ExternalInput")
    IND = nc.dram_tensor("ind", [nt_pad, GPC], E4, kind="ExternalInput")
    WGM = nc.dram_tensor("wgm", [128, 512], BF, kind="ExternalInput")
    WGR = nc.dram_tensor("wgr", [7, 512], BF, kind="ExternalInput")
    WNM = nc.dram_tensor("wnm", [128, 512], BF, kind="ExternalInput")
    WNR = nc.dram_tensor("wnr", [7, 512], BF, kind="ExternalInput")
    W2 = nc.dram_tensor("w2", [128, N_POOL], BF, kind="ExternalInput")
    S1 = nc.dram_tensor("s1", [128, 320], F32, kind="ExternalOutput")

    with TileContext(nc) as tc:
        with (
            tc.tile_pool(name="consts", bufs=1) as consts,
            tc.tile_pool(name="xin", bufs=3) as xin,
            tc.tile_pool(name="xrin", bufs=3) as xrin,
            tc.tile_pool(name="iin", bufs=4) as iin,
            tc.tile_pool(name="rgp", bufs=6) as rgp,
            tc.tile_pool(name="h1p", bufs=3) as h1p,
            tc.tile_pool(name="e2p", bufs=4) as e2p,
            tc.tile_pool(name="eip", bufs=8) as eip,
            tc.tile_pool(name="outp", bufs=1) as outp,
            tc.tile_pool(name="ps_g", bufs=2, space="PSUM") as ps_g,
            tc.tile_pool(name="ps_n", bufs=3, space="PSUM") as ps_n,
            tc.tile_pool(name="ps_acc", bufs=1, space="PSUM") as ps_acc,
        ):
            # --- constants ---
            wgm = consts.tile([128, 512], BF)
            nc.sync.dma_start(out=wgm, in_=WGM[:, :])
            wgr = consts.tile([7, 512], BF)
            nc.sync.dma_start(out=wgr, in_=WGR[:, :])
            wnm = consts.tile([128, 512], BF)
            nc.sync.dma_start(out=wnm, in_=WNM[:, :])
            wnr = consts.tile([7, 512], BF)
            nc.sync.dma_start(out=wnr, in_=WNR[:, :])
            w2s = consts.tile([128, N_POOL], BF)
            nc.sync.dma_start(out=w2s, in_=W2[:, :])
            w2n = consts.tile([128, N_POOL], BF)
            nc.vector.tensor_scalar_mul(w2n, w2s, -1.0)
            zs = consts.tile([128, 128], BF)
            nc.vector.memset(zs, 0.0)
            zm = consts.tile([128, 512], BF)
            nc.vector.memset(zm, 0.0)

            # --- persistent accumulator bank [128, 512] f32 ---
            # [0:64, 0:256]   pooled numerators: kh = slice*64 + partition,
            #                 slice j8 at cols j8*32:(j8+1)*32
            # [0:2, 256:320]  denominators: k-half m at cols 256+32m
            # [:, 384:416]    score scratch, 2-slot ping-pong of 16 cols
            acc = ps_acc.tile([128, 512], F32)
            nc.tensor.matmul(acc, zs, zm, start=True, stop=False, skip_group_check=True)

            st: list[dict] = [dict() for _ in range(T)]

            def relu_to(eng, out, in_):
                if eng == "S":
                    nc.scalar.activation(out, in_, Relu)
                else:
                    nc.vector.tensor_scalar_max(out, in_, 0.0)

            def emit_gate_pair(t, pair):
                s = st[t]
                if "rg" not in s:
                    s["rg"] = [None, None]
                pg = ps_g.tile([128, 2, 512], F32, tag="pg")
                for j in range(2):
                    k = 2 * pair + j
                    nc.tensor.matmul(
                        pg[:, j, :],
                        wgm[:, k * 128 : k * 128 + 128],
                        s["xm"],
                        start=True,
                        stop=False,
                    )
                    nc.tensor.matmul(
                        pg[:, j, :],
                        wgr[:, k * 128 : k * 128 + 128],
                        s["xr"],
                        start=False,
                        stop=True,
                    )
                rg = rgp.tile([128, 2, 512], BF, tag="rg")
                relu_to(cfg["gA" if pair == 0 else "gB"], rg, pg)
                s["rg"][pair] = rg

            def emit_nn_block(t, b):
                s = st[t]
                if "h1" not in s:
                    h1t = h1p.tile([128, 4, 512], E4, tag="h1")
                    s["h1"] = h1t
                h1 = s["h1"]
                pn = ps_n.tile([128, 512], F32, tag="pn")
                nc.tensor.matmul(
                    pn,
                    s["xm"][:, b * 128 : b * 128 + 128],
                    wnm,
                    start=True,
                    stop=False,
                )
                nc.tensor.matmul(
                    pn,
                    s["xr"][:, b * 128 : b * 128 + 128],
                    wnr,
                    start=False,
                    stop=True,
                )
                relu_to(cfg[f"b{b}"], h1[:, b, :], pn)

            def emit_scores(t, pair, neg=False):
                s = st[t]
                reg = 384 + 16 * (t % 2)
                w2 = w2n if neg else w2s
                for j in range(2):
                    k = 2 * pair + j
                    for i in range(4):
                        nc.tensor.matmul(
                            acc[:, reg + 4 * i + k : reg + 4 * i + k + 1],
                            s["rg"][pair][:, j, i * 128 : i * 128 + 128],
                            w2[:, k : k + 1],
                            start=False,
                            stop=False,
                            skip_group_check=True,
                        )

            def emit_exp_half(t, m):
                # exp for head-pair m (k = 2m, 2m+1) + its e*ind + slot clear
                s = st[t]
                reg = 384 + 16 * (t % 2)
                if "e2" not in s:
                    # layout [p, blk, k_padded16]: 16B blk stride for DR lhsT
                    e2t = e2p.tile([128, 4, 16], E4, tag="e2")
                    s["e2"] = e2t
                    s["ei"] = [None, None]
                e2 = s["e2"]
                nc.scalar.activation(
                    e2[:, :, 2 * m : 2 * m + 2],
                    acc[:, reg : reg + 16].rearrange("p (i k) -> p i k", k=N_POOL)[
                        :, :, 2 * m : 2 * m + 2
                    ],
                    Exp,
                )
                # e*ind on gpsimd (SBUF-only engine)
                ei = eip.tile([128, 2, 4, GPC], E4, tag="ei")
                nc.gpsimd.tensor_tensor(
                    ei,
                    s["i4"][:, None, :, :].to_broadcast([128, 2, 4, GPC]),
                    e2[:, :, 2 * m : 2 * m + 2]
                    .rearrange("p b k -> p k b")[:, :, :, None]
                    .to_broadcast([128, 2, 4, GPC]),
                    Mult,
                )
                s["ei"][m] = ei
                # recycle this head-pair's score cols for tile t+2
                if cfg.get("neg_clear"):
                    emit_scores(t, m, neg=True)
                else:
                    nc.vector.memset(
                        acc[:, reg : reg + 16].rearrange("p (i k) -> p k i", k=N_POOL)[
                            :, 2 * m : 2 * m + 2, :
                        ],
                        0.0,
                    )

            def emit_pool(t, last=False):
                s = st[t]
                h1 = s["h1"]
                for p in range(2):
                    for j8 in range(8):  # kh-slice: kh = j8*64 + partition
                        nc.tensor.matmul(
                            acc[0:64, j8 * 32 : j8 * 32 + 32],
                            h1[:, 2 * p : 2 * p + 2, j8 * 64 : j8 * 64 + 64],
                            s["ei"][j8 // 4][:, (j8 // 2) % 2, 2 * p : 2 * p + 2, :],
                            start=False,
                            stop=last and p == 1 and j8 == 7,
                            skip_group_check=True,
                            perf_mode=DR,
                        )
                    for m in range(2):  # den per k-half at cols 256+32m
                        nc.tensor.matmul(
                            acc[0:2, 256 + 32 * m : 288 + 32 * m],
                            s["e2"][:, 2 * p : 2 * p + 2, 2 * m : 2 * m + 2],
                            s["i4"][:, 2 * p : 2 * p + 2, :],
                            start=False,
                            stop=last and p == 1,
                            skip_group_check=True,
                            perf_mode=DR,
                        )

            for t in range(T):
                s = st[t]
                n0 = t * NT
                xm = xin.tile([128, NT], BF, tag="xm")
                nc.sync.dma_start(out=xm, in_=XM[:, n0 : n0 + NT])
                s["xm"] = xm
                xr = xrin.tile([7, NT], BF, tag="xr")
                nc.sync.dma_start(out=xr, in_=XR[:, n0 : n0 + NT])
                s["xr"] = xr
                i4t = iin.tile([128, 4, GPC], E4, tag="i4")
                nc.sync.dma_start(
                    out=i4t,
                    in_=IND[n0 : n0 + NT, :].rearrange("(blk p) b -> p blk b", p=128),
                )
                s["i4"] = i4t

                emit_gate_pair(t, 0)
                if t > 0:
                    emit_scores(t - 1, 1)
                    emit_exp_half(t - 1, 1)
                emit_gate_pair(t, 1)
                for b in range(4):
                    emit_nn_block(t, b)
                emit_scores(t, 0)
                emit_exp_half(t, 0)
                if t > 1:
                    emit_pool(t - 2)

            # epilogue
            emit_scores(T - 1, 1)
            emit_exp_half(T - 1, 1)
            emit_pool(T - 2)
            emit_pool(T - 1, last=True)

            s1_sb = outp.tile([128, 320], mybir.dt.float32)
            nc.vector.tensor_copy(s1_sb, acc[:, 0:320])
            nc.sync.dma_start(out=S1[:, :], in_=s1_sb)

    nc.compile()
    return nc


def _sim_makespan(nc) -> int:
    from concourse.timeline_sim import TimelineSim

    return int(TimelineSim(nc).simulate())


def kernel(**inputs) -> np.ndarray:
    global last_exec_time_ns, last_results, last_sim_ns
    import os

    x = np.asarray(inputs["x"], dtype=np.float32)  # [N, 134]
    batch = np.asarray(inputs["batch"]).astype(np.int64)  # [N], sorted
    gate_w1 = np.asarray(inputs["gate_w1"], dtype=np.float32)  # [4,134,128]
    gate_b1 = np.asarray(inputs["gate_b1"], dtype=np.float32)  # [4,128]
    gate_w2 = np.asarray(inputs["gate_w2"], dtype=np.float32)  # [4,128]
    nn_w1 = np.asarray(inputs["nn_w1"], dtype=np.float32)  # [4,134,128]
    nn_b1 = np.asarray(inputs["nn_b1"], dtype=np.float32)  # [4,128]
    nn_w2 = np.asarray(inputs["nn_w2"], dtype=np.float32)  # [4,128,128]
    nn_b2 = np.asarray(inputs["nn_b2"], dtype=np.float32)  # [4,128]

    N = x.shape[0]
    B = N_GRAPHS

    counts = np.bincount(batch, minlength=B)
    bounds = np.concatenate([[0], np.cumsum(counts)])
    core_start = bounds[np.arange(NCORES + 1) * GPC]
    shard_sizes = np.diff(core_start)
    nt_pad = int(-(-max(int(shard_sizes.max()), 1) // NT) * NT)

    # --- weights, [f, k*H] layout with bias row ---
    def pack_w(w1, b1):
        main = np.ascontiguousarray(
            w1[:, :128, :].transpose(1, 0, 2).reshape(128, 512)
        ).astype(BF16)
        rem = np.zeros((7, 512), dtype=BF16)
        rem[:6] = w1[:, 128:134, :].transpose(1, 0, 2).reshape(6, 512).astype(BF16)
        rem[6] = b1.reshape(512).astype(BF16)
        return main, rem

    wgm_h, wgr_h = pack_w(gate_w1, gate_b1)
    wnm_h, wnr_h = pack_w(nn_w1, nn_b1)
    w2_h = np.ascontiguousarray(gate_w2.T).astype(BF16)  # [128, 4]

    in_maps = []
    for c in range(NCORES):
        sN, eN = int(core_start[c]), int(core_start[c + 1])
        n = eN - sN
        xm = np.zeros((128, nt_pad), dtype=BF16)
        xm[:, :n] = x[sN:eN, :128].T.astype(BF16)
        xr = np.zeros((7, nt_pad), dtype=BF16)
        xr[:6, :n] = x[sN:eN, 128:134].T.astype(BF16)
        xr[6, :n] = 1.0
        ind = np.zeros((nt_pad, GPC), dtype=F8)
        if n > 0:
            ind[np.arange(n), batch[sN:eN] - c * GPC] = 1.0
        in_maps.append(
            {
                "xm": xm,
                "xr": xr,
                "ind": ind,
                "wgm": wgm_h,
                "wgr": wgr_h,
                "wnm": wnm_h,
                "wnr": wnr_h,
                "w2": w2_h,
            }
        )

    if nt_pad not in _cache:
        nc = _build(nt_pad)
        _cache[nt_pad] = (nc, _sim_makespan(nc))
    nc, last_sim_ns = _cache[nt_pad]

    trace = bool(os.environ.get("TRN_BASS_TRACE"))
    try:
        res = run_bass_kernel_spmd(
            nc, in_maps, core_ids=list(range(NCORES)), trace=trace
        )
    except ModuleNotFoundError:
        res = run_bass_kernel_spmd(
            nc, in_maps, core_ids=list(range(NCORES)), trace=False
        )
    last_exec_time_ns = res.exec_time_ns
    last_results = res

    # --- host-side finish (f32) ---
    pooled = np.zeros((NCORES, GPC, N_POOL, DIM_HID), np.float32)
    dens = np.zeros((NCORES, GPC, N_POOL), np.float32)
    for c in range(NCORES):
        raw = np.asarray(res.results[c]["s1"], np.float32)  # [128, 320]
        num = raw[0:64, 0:256].reshape(64, 8, 32)  # [p, j8, g], kh = j8*64+p
        den = raw[0:2, 256:320].reshape(2, 2, 32)  # [r, m, g] -> k = 2m + r
        kh = num.transpose(1, 0, 2).reshape(512, 32)  # [kh, g]
        pooled[c] = kh.reshape(N_POOL, DIM_HID, GPC).transpose(2, 0, 1)  # [g, k, h]
        dens[c] = den.transpose(2, 1, 0).reshape(GPC, N_POOL)  # [g, k=2m+r]
    den_safe = np.where(dens == 0.0, 1.0, dens)
    g1 = pooled / den_safe[..., None]
    out = np.einsum("cgkh,khd->cgkd", g1, nn_w2) + nn_b2
    nonempty = (counts.reshape(NCORES, GPC) > 0).astype(np.float32)
    out *= nonempty[:, :, None, None]
    ctx = out.reshape(B, N_POOL * DIM_EMB)

    extras = [
        np.asarray(inputs[k], dtype=np.float32)
        for k in [
            "n_nodes",
            "Omegas",
            "Phis",
            "Lambdas",
            "Omegas_norm",
            "Phis_norm",
            "Lambdas_norm",
        ]
    ]
    return np.concatenate([ctx] + extras, axis=1).astype(np.float32)


# revision 65
# speedup vs baseline: 1.4641x; 1.0499x over previous
"""Trainium2 Bass kernel for nn_ContextEncoder (4-head GlobalAttention pooling).

Strategy (v3, hardware-legal):
  - 8 contiguous graph-shards (batch sorted) -> data-parallel, no collectives.
  - Main x@W1 matmuls in bf16, two passes (features 0:128, then the 6
    remainder features + bias row as a 7-row pass).
  - Scores via the data-stationary trick: stationary = relu(gate hidden)
    block [128h x 128n], moving = a single w2 column -> out [128 nodes, 1].
    Nearly free on the PE (cost scales with moving free size).
  - Softmax denominators and gated segment-sum pooling accumulate in ONE
    persistent PSUM bank across all tiles (start=False matmuls onto a
    one-time-zeroed bank).  Scores live in the same bank in a 2-slot
    ping-pong region, recycled by PE matmuls with negated w2 (exact
    cancellation), so no engine has to memset PSUM.
  - Pooling contracts 256 nodes per pass via fp8 DoubleRow (stationary =
    h1 block-pair fp8, moving = e*ind block-pair fp8).  DoubleRow
    destinations must start at partition 0, so the accumulator keeps
    kh = slice*64 + partition with only partitions 0:64 used.
  - exp is split per head-pair so each PE->scalar score roundtrip overlaps
    the tile boundary; e*ind products run on gpsimd (SBUF-only there).
  - gate relus on the scalar engine, nn relus on DVE, e*ind on gpsimd:
    scalar ~2.4us, DVE ~2.6us, Pool ~1.3us, PE ~3.6us per 512-node tile.
  - Host finishes with the nn_w2 matmul (commutes with the segment sum)
    and the softmax normalization, in f32.
"""

import sys

sys.path.insert(0, "/opt/trn_rl_repo")

import numpy as np
import ml_dtypes

import concourse.bass as bass
import concourse.bacc as bacc
import concourse.mybir as mybir
from concourse.tile import TileContext
from concourse.bass_utils import run_bass_kernel_spmd

BF16 = ml_dtypes.bfloat16
F8 = ml_dtypes.float8_e4m3

N_POOL = 4
DIM_EMB = 128
DIM_HID = 128
FIRST_DIM = 134
N_GRAPHS = 256
NCORES = 8
GPC = N_GRAPHS // NCORES  # graphs per core
NT = 512  # nodes per tile

_cache: dict = {}

last_exec_time_ns = None
last_results = None
last_sim_ns = None

DR = mybir.MatmulPerfMode.DoubleRow
Relu = mybir.ActivationFunctionType.Relu
Exp = mybir.ActivationFunctionType.Exp
Mult = mybir.AluOpType.mult
Max = mybir.AluOpType.max

# engine assignment for balance-critical ops: "S"=scalar, "V"=DVE
# (gpsimd cannot touch PSUM on TRN2, so PSUM-reading relus are S/V only)
CFG = {
    "gA": "S", "gB": "S",
    "b0": "V", "b1": "V", "b2": "V", "b3": "V",
    "neg_clear": True,
}


def _build(nt_pad: int, cfg: dict | None = None):
    cfg = dict(CFG if cfg is None else cfg)
    F32 = mybir.dt.float32
    BF = mybir.dt.bfloat16
    E4 = mybir.dt.float8e4
    T = nt_pad // NT

    nc = bacc.Bacc("TRN2", target_bir_lowering=False, debug=False, num_devices=NCORES)

    XM = nc.dram_tensor("xm", [128, nt_pad], BF, kind="ExternalInput")
    XR = nc.dram_tensor("xr", [7, nt_pad], BF, kind="ExternalInput")
    IND = nc.dram_tensor("ind", [nt_pad, GPC], E4, kind="ExternalInput")
    WGM = nc.dram_tensor("wgm", [128, 512], BF, kind="ExternalInput")
    WGR = nc.dram_tensor("wgr", [7, 512], BF, kind="ExternalInput")
    WNM = nc.dram_tensor("wnm", [128, 512], BF, kind="ExternalInput")
    WNR = nc.dram_tensor("wnr", [7, 512], BF, kind="ExternalInput")
    W2 = nc.dram_tensor("w2", [128, N_POOL], BF, kind="ExternalInput")
    S1 = nc.dram_tensor("s1", [128, 320], F32, kind="ExternalOutput")

    with TileContext(nc) as tc:
        with (
            tc.tile_pool(name="consts", bufs=1) as consts,
            tc.tile_pool(name="xin", bufs=3) as xin,
            tc.tile_pool(name="xrin", bufs=3) as xrin,
            tc.tile_pool(name="iin", bufs=4) as iin,
            tc.tile_pool(name="rgp", bufs=6) as rgp,
            tc.tile_pool(name="h1p", bufs=3) as h1p,
            tc.tile_pool(name="e2p", bufs=4) as e2p,
            tc.tile_pool(name="eip", bufs=8) as eip,
            tc.tile_pool(name="outp", bufs=1) as outp,
            tc.tile_pool(name="ps_g", bufs=2, space="PSUM") as ps_g,
            tc.tile_pool(name="ps_n", bufs=3, space="PSUM") as ps_n,
            tc.tile_pool(name="ps_acc", bufs=1, space="PSUM") as ps_acc,
        ):
            # --- constants (issued on the otherwise-idle gpsimd queue) ---
            wgm = consts.tile([128, 512], BF)
            nc.gpsimd.dma_start(out=wgm, in_=WGM[:, :])
            wgr = consts.tile([7, 512], BF)
            nc.gpsimd.dma_start(out=wgr, in_=WGR[:, :])
            wnm = consts.tile([128, 512], BF)
            nc.gpsimd.dma_start(out=wnm, in_=WNM[:, :])
            wnr = consts.tile([7, 512], BF)
            nc.gpsimd.dma_start(out=wnr, in_=WNR[:, :])
            w2s = consts.tile([128, N_POOL], BF)
            nc.gpsimd.dma_start(out=w2s, in_=W2[:, :])
            w2n = consts.tile([128, N_POOL], BF)
            nc.vector.tensor_scalar_mul(w2n, w2s, -1.0)
            zs = consts.tile([128, 128], BF)
            nc.vector.memset(zs, 0.0)
            zm = consts.tile([128, 512], BF)
            nc.vector.memset(zm, 0.0)

            # --- persistent accumulator bank [128, 512] f32 ---
            # [0:64, 0:256]   pooled numerators: kh = slice*64 + partition,
            #                 slice j8 at cols j8*32:(j8+1)*32
            # [0:2, 256:320]  denominators: k-half m at cols 256+32m
            # [:, 384:416]    score scratch, 2-slot ping-pong of 16 cols
            acc = ps_acc.tile([128, 512], F32)

            st: list[dict] = [dict() for _ in range(T)]

            def relu_to(eng, out, in_):
                if eng == "S":
                    nc.scalar.activation(out, in_, Relu)
                else:
                    nc.vector.tensor_scalar_max(out, in_, 0.0)

            def emit_gate_pair(t, pair):
                s = st[t]
                if "rg" not in s:
                    s["rg"] = [None, None]
                pg = ps_g.tile([128, 2, 512], F32, tag="pg")
                for j in range(2):
                    k = 2 * pair + j
                    nc.tensor.matmul(
                        pg[:, j, :],
                        wgm[:, k * 128 : k * 128 + 128],
                        s["xm"],
                        start=True,
                        stop=False,
                    )
                    nc.tensor.matmul(
                        pg[:, j, :],
                        wgr[:, k * 128 : k * 128 + 128],
                        s["xr"],
                        start=False,
                        stop=True,
                    )
                rg = rgp.tile([128, 2, 512], BF, tag="rg")
                relu_to(cfg["gA" if pair == 0 else "gB"], rg, pg)
                s["rg"][pair] = rg

            def emit_nn_block(t, b):
                s = st[t]
                if "h1" not in s:
                    h1t = h1p.tile([128, 4, 512], E4, tag="h1")
                    s["h1"] = h1t
                h1 = s["h1"]
                pn = ps_n.tile([128, 512], F32, tag="pn")
                nc.tensor.matmul(
                    pn,
                    s["xm"][:, b * 128 : b * 128 + 128],
                    wnm,
                    start=True,
                    stop=False,
                )
                nc.tensor.matmul(
                    pn,
                    s["xr"][:, b * 128 : b * 128 + 128],
                    wnr,
                    start=False,
                    stop=True,
                )
                relu_to(cfg[f"b{b}"], h1[:, b, :], pn)

            def emit_scores(t, pair, neg=False):
                s = st[t]
                reg = 384 + 16 * (t % 2)
                w2 = w2n if neg else w2s
                for j in range(2):
                    k = 2 * pair + j
                    for i in range(4):
                        nc.tensor.matmul(
                            acc[:, reg + 4 * i + k : reg + 4 * i + k + 1],
                            s["rg"][pair][:, j, i * 128 : i * 128 + 128],
                            w2[:, k : k + 1],
                            start=False,
                            stop=False,
                            skip_group_check=True,
                        )

            def emit_exp_half(t, m):
                # exp for head-pair m (k = 2m, 2m+1) + its e*ind + slot clear
                s = st[t]
                reg = 384 + 16 * (t % 2)
                if "e2" not in s:
                    # layout [p, blk, k_padded16]: 16B blk stride for DR lhsT
                    e2t = e2p.tile([128, 4, 16], E4, tag="e2")
                    s["e2"] = e2t
                    s["ei"] = [None, None]
                e2 = s["e2"]
                nc.scalar.activation(
                    e2[:, :, 2 * m : 2 * m + 2],
                    acc[:, reg : reg + 16].rearrange("p (i k) -> p i k", k=N_POOL)[
                        :, :, 2 * m : 2 * m + 2
                    ],
                    Exp,
                )
                # e*ind on gpsimd (SBUF-only engine)
                ei = eip.tile([128, 2, 4, GPC], E4, tag="ei")
                nc.gpsimd.tensor_tensor(
                    ei,
                    s["i4"][:, None, :, :].to_broadcast([128, 2, 4, GPC]),
                    e2[:, :, 2 * m : 2 * m + 2]
                    .rearrange("p b k -> p k b")[:, :, :, None]
                    .to_broadcast([128, 2, 4, GPC]),
                    Mult,
                )
                s["ei"][m] = ei

            def emit_clear(t, m):
                # recycle head-pair m's score cols (consumed by exp) for t+2
                if cfg.get("neg_clear"):
                    emit_scores(t, m, neg=True)
                else:
                    reg = 384 + 16 * (t % 2)
                    nc.vector.memset(
                        acc[:, reg : reg + 16].rearrange("p (i k) -> p k i", k=N_POOL)[
                            :, 2 * m : 2 * m + 2, :
                        ],
                        0.0,
                    )

            def emit_pool(t, last=False):
                s = st[t]
                h1 = s["h1"]
                for p in range(2):
                    for j8 in range(8):  # kh-slice: kh = j8*64 + partition
                        nc.tensor.matmul(
                            acc[0:64, j8 * 32 : j8 * 32 + 32],
                            h1[:, 2 * p : 2 * p + 2, j8 * 64 : j8 * 64 + 64],
                            s["ei"][j8 // 4][:, (j8 // 2) % 2, 2 * p : 2 * p + 2, :],
                            start=False,
                            stop=last and p == 1 and j8 == 7,
                            skip_group_check=True,
                            perf_mode=DR,
                        )
                    for m in range(2):  # den per k-half at cols 256+32m
                        nc.tensor.matmul(
                            acc[0:2, 256 + 32 * m : 288 + 32 * m],
                            s["e2"][:, 2 * p : 2 * p + 2, 2 * m : 2 * m + 2],
                            s["i4"][:, 2 * p : 2 * p + 2, :],
                            start=False,
                            stop=last and p == 1,
                            skip_group_check=True,
                            perf_mode=DR,
                        )

            def emit_dma(t):
                s = st[t]
                n0 = t * NT
                xm = xin.tile([128, NT], BF, tag="xm")
                nc.sync.dma_start(out=xm, in_=XM[:, n0 : n0 + NT])
                s["xm"] = xm
                xr = xrin.tile([7, NT], BF, tag="xr")
                nc.sync.dma_start(out=xr, in_=XR[:, n0 : n0 + NT])
                s["xr"] = xr
                i4t = iin.tile([128, 4, GPC], E4, tag="i4")
                nc.sync.dma_start(
                    out=i4t,
                    in_=IND[n0 : n0 + NT, :].rearrange("(blk p) b -> p blk b", p=128),
                )
                s["i4"] = i4t

            for t in range(T):
                emit_dma(t)
                if t == 0:
                    # zero the accumulator bank; zeros-stationary x real moving
                    # makes this PE op wait for the first DMA, so the PE's
                    # first-ever op isn't followed by a multi-us idle gap
                    # (which would reset the p-state ramp).
                    nc.tensor.matmul(
                        acc, zs, st[0]["xm"], start=True, stop=False,
                        skip_group_check=True,
                    )
                emit_gate_pair(t, 0)
                if t > 0:
                    emit_scores(t - 1, 1)
                    emit_exp_half(t - 1, 1)
                emit_gate_pair(t, 1)
                for b in range(4):
                    emit_nn_block(t, b)
                emit_scores(t, 0)
                emit_exp_half(t, 0)
                if t > 1:
                    emit_pool(t - 2)
                if t > 0:
                    emit_clear(t - 1, 0)
                    emit_clear(t - 1, 1)

            # epilogue
            emit_scores(T - 1, 1)
            emit_exp_half(T - 1, 1)
            emit_pool(T - 2)
            emit_pool(T - 1, last=True)

            s1_sb = outp.tile([128, 320], mybir.dt.float32)
            nc.vector.tensor_copy(s1_sb, acc[:, 0:320])
            nc.sync.dma_start(out=S1[:, :], in_=s1_sb)

    nc.compile()
    return nc


def _sim_makespan(nc) -> int:
    from concourse.timeline_sim import TimelineSim

    return int(TimelineSim(nc).simulate())


def kernel(**inputs) -> np.ndarray:
    global last_exec_time_ns, last_results, last_sim_ns
    import os

    x = np.asarray(inputs["x"], dtype=np.float32)  # [N, 134]
    batch = np.asarray(inputs["batch"]).astype(np.int64)  # [N], sorted
    gate_w1 = np.asarray(inputs["gate_w1"], dtype=np.float32)  # [4,134,128]
    gate_b1 = np.asarray(inputs["gate_b1"], dtype=np.float32)  # [4,128]
    gate_w2 = np.asarray(inputs["gate_w2"], dtype=np.float32)  # [4,128]
    nn_w1 = np.asarray(inputs["nn_w1"], dtype=np.float32)  # [4,134,128]
    nn_b1 = np.asarray(inputs["nn_b1"], dtype=np.float32)  # [4,128]
    nn_w2 = np.asarray(inputs["nn_w2"], dtype=np.float32)  # [4,128,128]
    nn_b2 = np.asarray(inputs["nn_b2"], dtype=np.float32)  # [4,128]

    N = x.shape[0]
    B = N_GRAPHS

    counts = np.bincount(batch, minlength=B)
    bounds = np.concatenate([[0], np.cumsum(counts)])
    core_start = bounds[np.arange(NCORES + 1) * GPC]
    shard_sizes = np.diff(core_start)
    nt_pad = int(-(-max(int(shard_sizes.max()), 1) // NT) * NT)

    # --- weights, [f, k*H] layout with bias row ---
    def pack_w(w1, b1):
        main = np.ascontiguousarray(
            w1[:, :128, :].transpose(1, 0, 2).reshape(128, 512)
        ).astype(BF16)
        rem = np.zeros((7, 512), dtype=BF16)
        rem[:6] = w1[:, 128:134, :].transpose(1, 0, 2).reshape(6, 512).astype(BF16)
        rem[6] = b1.reshape(512).astype(BF16)
        return main, rem

    wgm_h, wgr_h = pack_w(gate_w1, gate_b1)
    wnm_h, wnr_h = pack_w(nn_w1, nn_b1)
    w2_h = np.ascontiguousarray(gate_w2.T).astype(BF16)  # [128, 4]

    in_maps = []
    for c in range(NCORES):
        sN, eN = int(core_start[c]), int(core_start[c + 1])
        n = eN - sN
        xm = np.zeros((128, nt_pad), dtype=BF16)
        xm[:, :n] = x[sN:eN, :128].T.astype(BF16)
        xr = np.zeros((7, nt_pad), dtype=BF16)
        xr[:6, :n] = x[sN:eN, 128:134].T.astype(BF16)
        xr[6, :n] = 1.0
        ind = np.zeros((nt_pad, GPC), dtype=F8)
        if n > 0:
            ind[np.arange(n), batch[sN:eN] - c * GPC] = 1.0
        in_maps.append(
            {
                "xm": xm,
                "xr": xr,
                "ind": ind,
                "wgm": wgm_h,
                "wgr": wgr_h,
                "wnm": wnm_h,
                "wnr": wnr_h,
                "w2": w2_h,
            }
        )

    if nt_pad not in _cache:
        nc = _build(nt_pad)
        _cache[nt_pad] = (nc, _sim_makespan(nc))
    nc, last_sim_ns = _cache[nt_pad]

    trace = bool(os.environ.get("TRN_BASS_TRACE"))
    try:
        res = run_bass_kernel_spmd(
            nc, in_maps, core_ids=list(range(NCORES)), trace=trace
        )
    except ModuleNotFoundError:
        res = run_bass_kernel_spmd(
            nc, in_maps, core_ids=list(range(NCORES)), trace=False
        )
    last_exec_time_ns = res.exec_time_ns
    last_results = res

    # --- host-side finish (f32) ---
    pooled = np.zeros((NCORES, GPC, N_POOL, DIM_HID), np.float32)
    dens = np.zeros((NCORES, GPC, N_POOL), np.float32)
    for c in range(NCORES):
        raw = np.asarray(res.results[c]["s1"], np.float32)  # [128, 320]
        num = raw[0:64, 0:256].reshape(64, 8, 32)  # [p, j8, g], kh = j8*64+p
        den = raw[0:2, 256:320].reshape(2, 2, 32)  # [r, m, g] -> k = 2m + r
        kh = num.transpose(1, 0, 2).reshape(512, 32)  # [kh, g]
        pooled[c] = kh.reshape(N_POOL, DIM_HID, GPC).transpose(2, 0, 1)  # [g, k, h]
        dens[c] = den.transpose(2, 1, 0).reshape(GPC, N_POOL)  # [g, k=2m+r]
    den_safe = np.where(dens == 0.0, 1.0, dens)
    g1 = pooled / den_safe[..., None]
    out = np.einsum("cgkh,khd->cgkd", g1, nn_w2) + nn_b2
    nonempty = (counts.reshape(NCORES, GPC) > 0).astype(np.float32)
    out *= nonempty[:, :, None, None]
    ctx = out.reshape(B, N_POOL * DIM_EMB)

    extras = [
        np.asarray(inputs[k], dtype=np.float32)
        for k in [
            "n_nodes",
            "Omegas",
            "Phis",
            "Lambdas",
            "Omegas_norm",
            "Phis_norm",
            "Lambdas_norm",
        ]
    ]
    return np.concatenate([ctx] + extras, axis=1).astype(np.float32)
